# revision 8
# baseline (speedup 1.0000x reference)
"""Trainium2 Bass kernel for nn_Calculator_61993557950977.

Math: for each beta, k_beta = floor(1/(1-(1-1/beta)) - 1)  (== floor(beta-1)
up to f32 rounding).  The reference's [B, dim] masked reductions collapse to

    c_j = #{b : k_beta_b > j}             (reverse cumulative histogram)
    d_j = sum_b [k_beta_b > j] * log(k_beta_b)

    ixt   = sum_j gamma_j * (d_j - log(j+1) * c_j)
    n_I   = sum_j gamma_j * c_j
    G     = sum_j gamma_j * log(lambda_j) * c_j
    H     = sum_j gamma_j * log1p(-lambda_j) * c_j

(the reference's log-ratio telescopes to log(k_beta) - log(j+1)).

On device, with j = 128*q + s (q in [0,32), s in [0,128)) and per-beta
(qb, rb) = divmod(k_beta, 128), a single transposed-orientation PSUM
accumulation produces everything:

    stationary[b, 1+s] = (s < rb_b), stationary[b, 0] = 1      [128, 128] bf16
    moving[b, :] = [onehot(qb) | onehot*lk_hi | onehot*lk_lo]  [128, 96] bf16

    psum[0,   q(+32k)] = hist[q] / histlog limbs               (ones row)
    psum[1+s, q]       = Pc[q, s] = #{b: qb=q, rb>s}
    psum[1+s, 32+q..]  = Pd limbs = sum lk*[qb=q][rb>s]

(lk = log(k_beta) split into bf16 hi+lo limbs so PE products stay exact in
f32 PSUM).  The j-space dot products then run with 128 partitions x 32 free
(4x less vector time than the q-major orientation), giving [128, 6] partial
sums; the host combines per-core partials (suffix sums of hist/histlog +
a handful of dots with table rowsums).

Batch (8192) is sharded 1024 per core across 8 cores, 8 tiles of 128.
Index grids are generated on device via iota; the only inputs are betas
(+identity for the PE transpose) and the transposed gamma/lambda/log(j+1)
tables.
"""

import os
import sys

for _p in ("/opt/trn_rl_repo",):
    if os.path.isdir(_p) and _p not in sys.path:
        sys.path.insert(0, _p)

import numpy as np

# Module constants from the reference nn.Module
IXY = 1.0
HX = 10.0
ALPHA = 2.0
C = 1.0
DIM = 4096
B = 8192

N_CORES = 8
BS = B // N_CORES          # betas per core
NT = BS // 128             # 8 batch tiles of 128 per core
NQ = 32                    # coarse bins  (DIM = NQ * GR)
GR = 128                   # fine bins per coarse bin
NVT = 4                    # step-mask tiles built on vector (rest on gpsimd)

_CACHE = {}


def _build_nc(surgery=True):
    import concourse.bacc as bacc
    import concourse.bass as bass
    import concourse.tile as tile
    from concourse import mybir

    f32 = mybir.dt.float32
    i32 = mybir.dt.int32
    bf16 = mybir.dt.bfloat16
    Alu = mybir.AluOpType
    ACT = mybir.ActivationFunctionType
    AX = mybir.AxisListType

    nc = bacc.Bacc("TRN2", target_bir_lowering=False, debug=False)

    # bin: [8,136] = betas rows | 8x8 identity
    bin_t = nc.dram_tensor("bin", [8, GR + 8], f32, kind="ExternalInput")
    # tb: [128,98] = gammaT | lambdaT | lnjT | 0.0 col | 1.0 col (row 0 of the
    # tables is zero: it pairs with the all-ones stationary column)
    tb_t = nc.dram_tensor("tb", [GR, 3 * NQ + 2], f32, kind="ExternalInput")
    o6_t = nc.dram_tensor("o6", [GR, 6], f32, kind="ExternalOutput")
    orow_t = nc.dram_tensor("orow", [1, 3 * NQ], f32, kind="ExternalOutput")

    def with_mid(ap, pair):
        # [P, ...] -> [P, pair, ...] inserting a (stride, size) dim after P
        return bass.AP(tensor=ap.tensor, offset=ap.offset,
                       ap=[ap.ap[0], pair] + list(ap.ap[1:]))

    def bc_mid(ap, n):
        return with_mid(ap, [0, n])

    def bc_last(ap, n):
        # [P, F] -> [P, F, n] with stride-0 last dim
        return bass.AP(tensor=ap.tensor, offset=ap.offset,
                       ap=[ap.ap[0], ap.ap[1], [0, n]])

    in_dmas = []
    with tile.TileContext(nc) as tc:
        with tc.tile_pool(name="sb", bufs=1) as sb, \
             tc.tile_pool(name="ps", bufs=1, space="PSUM") as ps:
            # ---- inputs (two parallel HWDGE queues: sync + scalar) ----
            bin8 = sb.tile([8, GR + 8], f32)
            in_dmas.append(nc.sync.dma_start(out=bin8, in_=bin_t[:, :]))
            tb = sb.tile([GR, 3 * NQ + 2], f32)
            in_dmas.append(nc.scalar.dma_start(out=tb, in_=tb_t[:, :]))

            beta8 = bin8[:, 0:GR]
            id8 = bin8[:, GR:GR + 8]
            gT = tb[:, 0:NQ]
            lT = tb[:, NQ:2 * NQ]
            lnjT = tb[:, 2 * NQ:3 * NQ]
            zc = tb[:, 3 * NQ:3 * NQ + 1]       # 0.0 col
            oc = tb[:, 3 * NQ + 1:3 * NQ + 2]   # 1.0 col

            # ---- index grids via iota (gpsimd, no deps) ----
            ci = sb.tile([GR, NQ + GR], i32)
            iq_i = ci[:, 0:NQ]
            ir_i = ci[:, NQ:]                   # values -1..126
            nc.gpsimd.iota(iq_i, pattern=[[1, NQ]], base=0,
                           channel_multiplier=0)
            nc.gpsimd.iota(ir_i, pattern=[[1, GR]], base=-1,
                           channel_multiplier=0)

            # ---- transpose betas to [128, NT] via the tensor engine ----
            beta_ps = ps.tile([GR, 8], f32)
            nc.tensor.transpose(beta_ps, beta8, id8)

            # ---- per-beta prep ([128, NT]) ----
            # k_beta = floor(beta - 1) via RNE int writeback of (beta - 1.5).
            kbi = sb.tile([128, NT], i32)
            rbi = sb.tile([128, NT], i32)
            qbi = sb.tile([128, NT], i32)
            lk = sb.tile([128, NT], f32)
            limb = sb.tile([128, NT, 2], bf16)
            S = sb.tile([128, NT, GR], bf16)
            M = sb.tile([128, NT, 3 * NQ], bf16)
            with tc.high_priority():
                nc.vector.tensor_scalar(kbi, beta_ps, 1.5, None,
                                        op0=Alu.subtract)
                nc.vector.tensor_scalar(rbi, kbi, 127, None,
                                        op0=Alu.bitwise_and)
                nc.vector.tensor_scalar(qbi, kbi, 7, None,
                                        op0=Alu.arith_shift_right)
                # stationary step masks: S[:, t, 1+s] = (s < rb), col 0 = ones
                # (two halves so the PE can start on tiles 0-3 earlier; Pool
                # cannot take a share: int compares fail its engine check)
                nc.vector.tensor_tensor(
                    S[:, 0:NVT, :], bc_mid(ir_i, NVT),
                    bc_last(rbi[:, 0:NVT], GR), op=Alu.is_lt)
                nc.vector.tensor_tensor(
                    S[:, NVT:NT, :], bc_mid(ir_i, NT - NVT),
                    bc_last(rbi[:, NVT:NT], GR), op=Alu.is_lt)
                # moving: onehot(qb) and lk-limb-scaled onehots
                nc.vector.tensor_tensor(M[:, :, 0:NQ], bc_mid(iq_i, NT),
                                        bc_last(qbi, NQ), op=Alu.is_equal)
                nc.scalar.activation(out=lk, in_=kbi, func=ACT.Ln, bias=zc)
                nc.scalar.activation(out=limb[:, :, 0], in_=kbi, func=ACT.Ln,
                                     bias=zc)          # hi limb (bf16 RNE)
                nc.vector.tensor_tensor(limb[:, :, 1], lk, limb[:, :, 0],
                                        op=Alu.subtract)   # lo limb
                # M[:, t, 32+32*l+q] = onehot * limb_l  (both limbs at once)
                hi_sl = M[:, :, NQ:2 * NQ]
                o_dst = with_mid(hi_sl, [NQ, 2])
                o_src = bc_mid(M[:, :, 0:NQ], 2)
                l_src = bass.AP(tensor=limb.tensor, offset=limb.offset,
                                ap=[limb.ap[0], [1, 2], [2, NT], [0, NQ]])
                nc.vector.tensor_tensor(o_dst, o_src, l_src, op=Alu.mult)

            # ---- single PSUM accumulation over the 8 batch tiles ----
            psum = ps.tile([GR, 3 * NQ], f32)
            for t in range(NT):
                nc.tensor.matmul(psum, S[:, t, :], M[:, t, :],
                                 start=(t == 0), stop=(t == NT - 1))

            # ---- weight tables [128, 4, NQ] (scalar+gpsimd, overlap) ----
            lnl = sb.tile([GR, NQ], f32)
            nc.scalar.activation(out=lnl, in_=lT, func=ACT.Ln, bias=zc)
            ln1m = sb.tile([GR, NQ], f32)
            nc.scalar.activation(out=ln1m, in_=lT, func=ACT.Ln, bias=oc,
                                 scale=-1.0)
            T4 = sb.tile([GR, 4, NQ], f32)
            nc.gpsimd.tensor_tensor(T4[:, 0, :], lnjT, gT, op=Alu.mult)
            nc.gpsimd.tensor_copy(T4[:, 1, :], gT)
            nc.gpsimd.tensor_tensor(T4[:, 2, :], lnl, gT, op=Alu.mult)
            nc.gpsimd.tensor_tensor(T4[:, 3, :], ln1m, gT, op=Alu.mult)

            # ---- dot products against Pc / Pd (vector reads PSUM) ----
            P6 = sb.tile([GR, 6, NQ], f32)
            nc.vector.tensor_tensor(P6[:, 0:4, :], T4,
                                    bc_mid(psum[:, 0:NQ], 4), op=Alu.mult)
            pd_sl = psum[:, NQ:2 * NQ]
            pd = with_mid(pd_sl, [NQ, 2])
            nc.vector.tensor_tensor(P6[:, 4:6, :], bc_mid(gT, 2), pd,
                                    op=Alu.mult)
            o6sb = sb.tile([GR, 6], f32)
            nc.vector.tensor_reduce(o6sb, P6, axis=AX.X, op=Alu.add)
            # row 0 of psum = [hist | histlog_hi | histlog_lo]
            orow = sb.tile([1, 3 * NQ], f32)
            nc.scalar.copy(orow, psum[0:1, :])

            nc.sync.dma_start(out=o6_t[:, :], in_=o6sb)
            nc.scalar.dma_start(out=orow_t[:, :], in_=orow)

    nc.compile()
    if surgery:
        _surgery(nc)
    return nc


def _surgery(nc):
    """Post-compile stream surgery:
    - drop the all-engine entry barrier from the main block (body ordering is
      fully semaphore-protected, and the entry/exit barriers each consume
      exactly what they produce on S[151]/S[152], so the exit barrier still
      works), letting engines enter the body immediately after NEFF init;
    - hoist the input DMA dispatches to the head of the body block so their
      doorbells ring before the scalar engine's ~2.6us of ACT table loads.
    """
    f = nc.m.functions[0]
    main = f.blocks[0]
    main.instructions = [
        i for i in main.instructions
        if type(i).__name__ not in ("InstMemset", "InstDrain",
                                    "InstEventSemaphore")]
    body = f.blocks[1]

    def is_input_dma(i):
        if type(i).__name__ != "InstDMACopy" or not i.ins:
            return False
        return getattr(i.ins[0], "memref", None) in ("bin", "tb")

    front = [i for i in body.instructions if is_input_dma(i)]
    rest = [i for i in body.instructions if not is_input_dma(i)]
    assert len(front) == 2, f"expected 2 input DMAs, found {len(front)}"
    body.instructions = front + rest


def _consts():
    lnj = np.log(np.arange(1, DIM + 1, dtype=np.float64)).astype(np.float32)
    return lnj


def run_device(betas, lambdas, gammas, trace=False):
    from concourse.bass_utils import run_bass_kernel_spmd

    if "nc" not in _CACHE:
        _CACHE["nc"] = _build_nc()
    nc = _CACHE["nc"]

    betas = np.ascontiguousarray(np.asarray(betas, dtype=np.float32).reshape(B))
    lambdas = np.asarray(lambdas, dtype=np.float32).reshape(DIM)
    gammas = np.asarray(gammas, dtype=np.float32).reshape(DIM)
    lnj = _consts()

    # transposed tables with the ones-row (s'=-1) slot zeroed
    def tshift(v, row0=0.0):
        out = np.full((GR, NQ), row0, np.float32)
        out[1:, :] = v.reshape(NQ, GR)[:, 0:GR - 1].T
        return out

    tb = np.concatenate([
        tshift(gammas), tshift(lambdas, row0=0.5), tshift(lnj),
        np.zeros((GR, 1), np.float32), np.ones((GR, 1), np.float32)],
        axis=1)
    tb = np.ascontiguousarray(tb)

    in_maps = []
    for i in range(N_CORES):
        bn = np.zeros((8, GR + 8), np.float32)
        bn[:, 0:GR] = betas[i * BS:(i + 1) * BS].reshape(8, GR)
        bn[:, GR:GR + 8] = np.eye(8, dtype=np.float32)
        in_maps.append({"bin": bn, "tb": tb})

    last_err = None
    res = None
    for _attempt in range(3):
        try:
            res = run_bass_kernel_spmd(nc, in_maps, core_ids=list(range(N_CORES)),
                                       trace=trace)
            break
        except Exception as e:  # transient device-recovery errors
            last_err = e
            res = None
    if res is None:
        raise last_err

    o6 = np.stack([np.asarray(r["o6"], dtype=np.float64) for r in res.results])
    orow = np.stack([np.asarray(r["orow"], dtype=np.float64).reshape(3 * NQ)
                     for r in res.results])
    hist = orow[:, 0:NQ]
    hlog = orow[:, NQ:2 * NQ] + orow[:, 2 * NQ:3 * NQ]
    Cq = np.cumsum(hist[:, ::-1], axis=1)[:, ::-1] - hist   # exclusive suffix
    Dq = np.cumsum(hlog[:, ::-1], axis=1)[:, ::-1] - hlog
    # beta-independent table rowsums (host, f64)
    g64 = gammas.astype(np.float64)
    l64 = lambdas.astype(np.float64)
    lnj64 = np.log(np.arange(1, DIM + 1, dtype=np.float64))
    rs_lnj = (g64 * lnj64).reshape(NQ, GR).sum(1)
    rs_g = g64.reshape(NQ, GR).sum(1)
    rs_lnl = (g64 * np.log(l64)).reshape(NQ, GR).sum(1)
    rs_ln1m = (g64 * np.log1p(-l64)).reshape(NQ, GR).sum(1)
    E2 = o6[:, :, 0].sum() + (Cq * rs_lnj).sum()
    Nn = o6[:, :, 1].sum() + (Cq * rs_g).sum()
    G = o6[:, :, 2].sum() + (Cq * rs_lnl).sum()
    H = o6[:, :, 3].sum() + (Cq * rs_ln1m).sum()
    E1 = (o6[:, :, 4] + o6[:, :, 5]).sum() + (Dq * rs_g).sum()
    sums = (E1, E2, Nn, G, H)
    return sums, res


def _finalize(E1, E2, Nn, G, H):
    ixt = E1 - E2
    n_I = Nn
    gm_term = np.exp(G / n_I)
    gm_comp = np.exp(H / n_I)
    exp_term = np.exp(2.0 * ixt / n_I)
    log_term = -n_I / 2.0 * np.log(gm_comp + exp_term * gm_term)
    ity = ixt + log_term
    rhs = 1.0 - ity / IXY
    lhs_1 = 1.0 - ixt / HX
    if lhs_1 < 0:
        lhs_1 = abs(lhs_1) * 20.0
    lhs = C * lhs_1 ** ALPHA
    return (np.asarray(np.float32(rhs)), np.asarray(np.float32(lhs)))


def kernel(betas, lambdas, gammas):
    sums, _ = run_device(betas, lambdas, gammas, trace=False)
    return _finalize(*sums)


# revision 14
# speedup vs baseline: 1.2973x; 1.2973x over previous
"""Trainium2 Bass kernel for nn_Calculator_61993557950977.

Math: for each beta, k_beta = floor(1/(1-(1-1/beta)) - 1)  (== floor(beta-1)
up to f32 rounding).  The reference's [B, dim] masked reductions collapse to

    c_j = #{b : k_beta_b > j}             (reverse cumulative histogram)
    d_j = sum_b [k_beta_b > j] * log(k_beta_b)

    ixt   = sum_j gamma_j * (d_j - log(j+1) * c_j)
    n_I   = sum_j gamma_j * c_j
    G     = sum_j gamma_j * log(lambda_j) * c_j
    H     = sum_j gamma_j * log1p(-lambda_j) * c_j

(the reference's log-ratio telescopes to log(k_beta) - log(j+1)).

On device, with j = 128*q + s (q in [0,32), s in [0,128)) and per-beta
(qb, rb) = divmod(k_beta, 128), a single transposed-orientation PSUM
accumulation over 8 batch tiles produces everything:

    stationary[b, 1+s] = (s < rb_b), stationary[b, 0] = 1      [128, 128] bf16
    moving[b, :] = [onehot(qb) | onehot*lk_hi | onehot*lk_lo]  [128, 96] bf16

    psum[0,   32k+q] = hist[q] / histlog limbs                 (ones row)
    psum[1+s, q]     = Pc[q, s] = #{b: qb=q, rb>s}
    psum[1+s, 32k+q] = Pd limbs = sum lk*[qb=q][rb>s]

(lk = log(k_beta) split into bf16 hi+lo limbs so PE products stay exact in
f32 PSUM).  The j-space dot products then run with 128 partitions x 32 free:
GP = gammaT*Pc once, then {lnjT, ln(lambda)T, ln1p(-lambda)T} * GP in one
3-block multiply, plus gammaT*Pd limbs; a [1,6] PE column-sum (ones
stationary) collapses partitions so a single 1-packet [1,102] DMA returns
the 6 dot sums + the hist/histlog row.  The host combines per-core partials
(suffix sums + a handful of dots with table rowsums).

Batch (8192) is sharded 1024 per core across 8 cores, 8 tiles of 128.
Index grids are int16 (2x DVE compare rate); all index math is int16.
"""

import os
import sys

for _p in ("/opt/trn_rl_repo",):
    if os.path.isdir(_p) and _p not in sys.path:
        sys.path.insert(0, _p)

import numpy as np

# Module constants from the reference nn.Module
IXY = 1.0
HX = 10.0
ALPHA = 2.0
C = 1.0
DIM = 4096
B = 8192

N_CORES = 8
BS = B // N_CORES          # betas per core
NT = BS // 128             # 8 batch tiles of 128 per core
NQ = 32                    # coarse bins  (DIM = NQ * GR)
GR = 128                   # fine bins per coarse bin
NVT = 4                    # step-mask tiles in the first (PE-feeding) half

_CACHE = {}


def _build_nc(surgery=True):
    import concourse.bacc as bacc
    import concourse.bass as bass
    import concourse.tile as tile
    from concourse import mybir

    f32 = mybir.dt.float32
    i16 = mybir.dt.int16
    bf16 = mybir.dt.bfloat16
    Alu = mybir.AluOpType
    ACT = mybir.ActivationFunctionType
    AX = mybir.AxisListType

    nc = bacc.Bacc("TRN2", target_bir_lowering=False, debug=False)

    # bin: [8,136] = betas rows | 8x8 identity
    bin_t = nc.dram_tensor("bin", [8, GR + 8], f32, kind="ExternalInput")
    # ci: [128,160] int16 = iq grid (0..31) | ir grid (-1..126)
    ci_t = nc.dram_tensor("ci", [GR, NQ + GR], i16, kind="ExternalInput")
    # tb: [128,98] = gammaT | lambdaT | 0.0 | 1.0 | lnjT   (rows shifted so
    # row 0 pairs with the all-ones stationary column and is zero)
    tb_t = nc.dram_tensor("tb", [GR, 3 * NQ + 2], f32, kind="ExternalInput")
    out_t = nc.dram_tensor("out", [1, 3 * NQ + 6], f32, kind="ExternalOutput")

    def with_mid(ap, pair):
        # [P, ...] -> [P, pair, ...] inserting a (stride, size) dim after P
        return bass.AP(tensor=ap.tensor, offset=ap.offset,
                       ap=[ap.ap[0], pair] + list(ap.ap[1:]))

    def bc_mid(ap, n):
        return with_mid(ap, [0, n])

    def bc_last(ap, n):
        # [P, F] -> [P, F, n] with stride-0 last dim
        return bass.AP(tensor=ap.tensor, offset=ap.offset,
                       ap=[ap.ap[0], ap.ap[1], [0, n]])

    with tile.TileContext(nc) as tc:
        with tc.tile_pool(name="sb", bufs=1) as sb, \
             tc.tile_pool(name="ps", bufs=1, space="PSUM") as ps:
            # ---- inputs (sync queue: bin+ci; scalar queue: tb) ----
            bin8 = sb.tile([8, GR + 8], f32)
            nc.sync.dma_start(out=bin8, in_=bin_t[:, :])
            ci = sb.tile([GR, NQ + GR], i16)
            nc.sync.dma_start(out=ci, in_=ci_t[:, :])
            # tb tile has 64 extra cols that the scalar engine fills with
            # ln(lambda)T / log1p(-lambda)T so the dot tables are contiguous
            tb = sb.tile([GR, 3 * NQ + 2 + 2 * NQ], f32)
            nc.scalar.dma_start(out=tb[:, 0:3 * NQ + 2], in_=tb_t[:, :])

            iq_i = ci[:, 0:NQ]
            ir_i = ci[:, NQ:]                   # values -1..126
            gT = tb[:, 0:NQ]
            lT = tb[:, NQ:2 * NQ]
            zc = tb[:, 2 * NQ:2 * NQ + 1]       # 0.0 col
            oc = tb[:, 2 * NQ + 1:2 * NQ + 2]   # 1.0 col (also PE-sum ones)
            t3sl = tb[:, 2 * NQ + 2:5 * NQ + 2]  # lnjT|lnl|ln1m, contiguous
            T3 = bass.AP(tensor=t3sl.tensor, offset=t3sl.offset,
                         ap=[t3sl.ap[0], [NQ, 3], [1, NQ]])
            lnl = tb[:, 3 * NQ + 2:4 * NQ + 2]
            ln1m = tb[:, 4 * NQ + 2:5 * NQ + 2]

            # ---- transpose betas to [128, NT] via the tensor engine ----
            beta_ps = ps.tile([GR, 8], f32)
            nc.tensor.transpose(beta_ps, bin8[:, 0:GR], bin8[:, GR:GR + 8])

            # ---- per-beta prep ([128, NT], int16) ----
            kh = sb.tile([128, NT], f32)
            kbi = sb.tile([128, NT], i16)
            rbi = sb.tile([128, NT], i16)
            qbi = sb.tile([128, NT], i16)
            lk = sb.tile([128, NT], f32)
            zcol = sb.tile([128, 1], f32)
            limb = sb.tile([128, NT, 2], bf16)
            S = sb.tile([128, NT, GR], bf16)
            M = sb.tile([128, 3, NT, NQ], bf16)   # block-major: oh|oh*hi|oh*lo
            with tc.high_priority():
                nc.vector.tensor_scalar(zcol, beta_ps[:, 0:1], 0.0, None,
                                        op0=Alu.mult)
                # k_beta = floor(beta-1) via RNE int writeback of (beta-1.5)
                # (two steps: int16 writeback is rejected for PSUM sources)
                nc.vector.tensor_scalar(kh, beta_ps, 1.0, None,
                                        op0=Alu.subtract)
                nc.vector.tensor_scalar(kbi, kh, 0.5, None,
                                        op0=Alu.subtract)
                nc.vector.tensor_scalar(rbi, kbi, 127, None,
                                        op0=Alu.bitwise_and)
                # q = floor(k/128) via RNE((beta-1)/128 - 0.5): int16 shifts
                # fail the ISA check, but k/128 is exact in f32
                nc.vector.tensor_scalar(qbi, kh, 1.0 / 128.0, 0.5,
                                        op0=Alu.mult, op1=Alu.subtract)
                # stationary step masks: S[:, t, 1+s] = (s < rb), col 0 = 1
                nc.vector.tensor_tensor(
                    S[:, 0:NVT, :], bc_mid(ir_i, NVT),
                    bc_last(rbi[:, 0:NVT], GR), op=Alu.is_lt)
                nc.vector.tensor_tensor(M[:, 0, :, :], bc_mid(iq_i, NT),
                                        bc_last(qbi, NQ), op=Alu.is_equal)
                nc.scalar.activation(out=lk, in_=kbi, func=ACT.Ln, bias=zcol)
                nc.vector.tensor_copy(limb[:, :, 0], lk)          # hi limb
                nc.vector.tensor_tensor(limb[:, :, 1], lk, limb[:, :, 0],
                                        op=Alu.subtract)          # lo limb
                # M[:, 1+l, t, q] = onehot * limb_l  (both limbs, contiguous)
                o_dst = M[:, 1:3, :, :]
                o_src = bc_mid(M[:, 0, :, :], 2)
                lf = limb[:, :, :]
                l_src = bass.AP(tensor=lf.tensor, offset=lf.offset,
                                ap=[lf.ap[0], [1, 2], [2, NT], [0, NQ]])
                nc.vector.tensor_tensor(o_dst, o_src, l_src, op=Alu.mult)
                nc.vector.tensor_tensor(
                    S[:, NVT:NT, :], bc_mid(ir_i, NT - NVT),
                    bc_last(rbi[:, NVT:NT], GR), op=Alu.is_lt)

            # ---- single PSUM accumulation over the 8 batch tiles ----
            psum = ps.tile([GR, 3 * NQ], f32)
            for t in range(NT):
                nc.tensor.matmul(psum, S[:, t, :], M[:, :, t, :],
                                 start=(t == 0), stop=(t == NT - 1))

            # ---- lambda log tables (scalar; de-prioritized vs lk path) ----
            with tc.high_priority(offset=-100000):
                nc.scalar.activation(out=lnl, in_=lT, func=ACT.Ln, bias=zc)
                nc.scalar.activation(out=ln1m, in_=lT, func=ACT.Ln, bias=oc,
                                     scale=-1.0)

            # ---- dot products against Pc / Pd (vector reads PSUM) ----
            # P6 blocks: 0=E2', 1=G', 2=H', 3=Nn' (=GP), 4:6=E1' limbs
            P6 = sb.tile([GR, 6, NQ], f32)
            GP = P6[:, 3, :]
            nc.vector.tensor_tensor(GP, gT, psum[:, 0:NQ], op=Alu.mult)
            nc.vector.tensor_tensor(P6[:, 0:3, :], T3, bc_mid(GP, 3),
                                    op=Alu.mult)
            pd = with_mid(psum[:, NQ:2 * NQ], [NQ, 2])
            nc.vector.tensor_tensor(P6[:, 4:6, :], bc_mid(gT, 2), pd,
                                    op=Alu.mult)
            o6sb = sb.tile([GR, 6], f32)
            nc.vector.tensor_reduce(o6sb, P6, axis=AX.X, op=Alu.add)
            # collapse partitions on the PE: psum6[0, c] = sum_p o6sb[p, c]
            psum6 = ps.tile([1, 6], f32)
            nc.tensor.matmul(psum6, oc, o6sb, start=True, stop=True)

            outsb = sb.tile([1, 3 * NQ + 6], f32)
            nc.vector.tensor_copy(outsb[:, 0:3 * NQ], psum[0:1, :])
            nc.vector.tensor_copy(outsb[:, 3 * NQ:3 * NQ + 6], psum6)
            nc.sync.dma_start(out=out_t[:, :], in_=outsb)

    nc.compile()
    if surgery:
        _surgery(nc)
    return nc


def _surgery(nc):
    """Post-compile stream surgery:
    - drop const-AP memsets and the all-engine entry barrier from the main
      block (body ordering is fully semaphore-protected; the entry/exit
      barriers each consume exactly what they produce on their semaphores,
      so the exit barrier still works);
    - hoist the input DMA dispatches to the head of the body block so their
      doorbells ring before the scalar engine's ACT table load;
    - drop the exit-block's leading DMA-completion waits (nothing on device
      consumes the output DMA; its semaphore is write-only) and the second
      exit barrier after the semaphore range-clear (the NEFF's own final
      all-engine rendezvous follows immediately).
    """
    f = nc.m.functions[0]
    main = f.blocks[0]
    main.instructions = [
        i for i in main.instructions
        if type(i).__name__ not in ("InstMemset", "InstDrain",
                                    "InstEventSemaphore")]
    body = f.blocks[1]

    def is_input_dma(i):
        if type(i).__name__ != "InstDMACopy" or not i.ins:
            return False
        return getattr(i.ins[0], "memref", None) in ("bin", "ci", "tb")

    front = [i for i in body.instructions if is_input_dma(i)]
    rest = [i for i in body.instructions if not is_input_dma(i)]
    assert len(front) == 3, f"expected 3 input DMAs, found {len(front)}"
    body.instructions = front + rest

    end = f.blocks[2]
    insts = list(end.instructions)
    i = 0
    while i < len(insts) and type(insts[i]).__name__ == "InstEventSemaphore":
        i += 1
    insts = insts[i:]
    isa = [j for j, x in enumerate(insts) if type(x).__name__ == "InstISA"]
    if isa:
        insts = insts[:isa[-1] + 1]
    end.instructions = insts


def run_device(betas, lambdas, gammas, trace=False):
    from concourse.bass_utils import run_bass_kernel_spmd

    if "nc" not in _CACHE:
        _CACHE["nc"] = _build_nc()
    nc = _CACHE["nc"]

    betas = np.ascontiguousarray(np.asarray(betas, dtype=np.float32).reshape(B))
    lambdas = np.asarray(lambdas, dtype=np.float32).reshape(DIM)
    gammas = np.asarray(gammas, dtype=np.float32).reshape(DIM)
    lnj = np.log(np.arange(1, DIM + 1, dtype=np.float64)).astype(np.float32)

    # transposed tables with the ones-row (s'=-1) slot zeroed
    def tshift(v, row0=0.0):
        out = np.full((GR, NQ), row0, np.float32)
        out[1:, :] = v.reshape(NQ, GR)[:, 0:GR - 1].T
        return out

    tb = np.concatenate([
        tshift(gammas), tshift(lambdas, row0=0.5),
        np.zeros((GR, 1), np.float32), np.ones((GR, 1), np.float32),
        tshift(lnj)], axis=1)
    tb = np.ascontiguousarray(tb)
    iq = np.broadcast_to(np.arange(NQ, dtype=np.int16), (GR, NQ))
    ir = np.broadcast_to(np.arange(-1, GR - 1, dtype=np.int16), (GR, GR))
    ci = np.ascontiguousarray(np.concatenate([iq, ir], axis=1))

    in_maps = []
    for i in range(N_CORES):
        bn = np.zeros((8, GR + 8), np.float32)
        bn[:, 0:GR] = betas[i * BS:(i + 1) * BS].reshape(8, GR)
        bn[:, GR:GR + 8] = np.eye(8, dtype=np.float32)
        in_maps.append({"bin": bn, "ci": ci, "tb": tb})

    last_err = None
    res = None
    for _attempt in range(3):
        try:
            res = run_bass_kernel_spmd(nc, in_maps, core_ids=list(range(N_CORES)),
                                       trace=trace)
            break
        except Exception as e:  # transient device-recovery errors
            last_err = e
            res = None
    if res is None:
        raise last_err

    o = np.stack([np.asarray(r["out"], dtype=np.float64).reshape(3 * NQ + 6)
                  for r in res.results])
    hist = o[:, 0:NQ]
    hlog = o[:, NQ:2 * NQ] + o[:, 2 * NQ:3 * NQ]
    d6 = o[:, 3 * NQ:3 * NQ + 6]     # [cores, 6]: E2' G' H' Nn' E1hi' E1lo'
    Cq = np.cumsum(hist[:, ::-1], axis=1)[:, ::-1] - hist   # exclusive suffix
    Dq = np.cumsum(hlog[:, ::-1], axis=1)[:, ::-1] - hlog
    # beta-independent table rowsums (host, f64)
    g64 = gammas.astype(np.float64)
    l64 = lambdas.astype(np.float64)
    lnj64 = np.log(np.arange(1, DIM + 1, dtype=np.float64))
    rs_lnj = (g64 * lnj64).reshape(NQ, GR).sum(1)
    rs_g = g64.reshape(NQ, GR).sum(1)
    rs_lnl = (g64 * np.log(l64)).reshape(NQ, GR).sum(1)
    rs_ln1m = (g64 * np.log1p(-l64)).reshape(NQ, GR).sum(1)
    E2 = d6[:, 0].sum() + (Cq * rs_lnj).sum()
    G = d6[:, 1].sum() + (Cq * rs_lnl).sum()
    H = d6[:, 2].sum() + (Cq * rs_ln1m).sum()
    Nn = d6[:, 3].sum() + (Cq * rs_g).sum()
    E1 = (d6[:, 4] + d6[:, 5]).sum() + (Dq * rs_g).sum()
    sums = (E1, E2, Nn, G, H)
    return sums, res


def _finalize(E1, E2, Nn, G, H):
    ixt = E1 - E2
    n_I = Nn
    gm_term = np.exp(G / n_I)
    gm_comp = np.exp(H / n_I)
    exp_term = np.exp(2.0 * ixt / n_I)
    log_term = -n_I / 2.0 * np.log(gm_comp + exp_term * gm_term)
    ity = ixt + log_term
    rhs = 1.0 - ity / IXY
    lhs_1 = 1.0 - ixt / HX
    if lhs_1 < 0:
        lhs_1 = abs(lhs_1) * 20.0
    lhs = C * lhs_1 ** ALPHA
    return (np.asarray(np.float32(rhs)), np.asarray(np.float32(lhs)))


def kernel(betas, lambdas, gammas):
    sums, _ = run_device(betas, lambdas, gammas, trace=False)
    return _finalize(*sums)
